# revision 3
# baseline (speedup 1.0000x reference)
"""Trainium2 Bass kernel for causal self-attention (B=4, T=2048, C=1024, H=16).

Sharding: 8 cores = 4 batches (data-parallel) x 2 head-groups (tensor-parallel,
8 heads each). Each core computes QKV for its heads, flash-style causal
attention, and a partial output projection over its half of the channels;
a pairwise ReduceScatter over tokens combines the two partials per batch.

Self-contained: hardcodes shapes; host side only slices/concats numpy arrays.
"""

import ml_dtypes
import numpy as np
from contextlib import ExitStack

import concourse.bass as bass
import concourse.tile as tile
from concourse import bacc, mybir
from concourse.bass_utils import run_bass_kernel_spmd
from concourse.masks import make_identity, make_upper_triangular

F32 = mybir.dt.float32
BF16 = mybir.dt.bfloat16
AF = mybir.ActivationFunctionType
ALU = mybir.AluOpType

B, T, C = 4, 2048, 1024
H, HD = 16, 64
G = 2                    # tensor-parallel head groups
HL = H // G              # heads per core (8)
FL = HL * HD             # local q/k/v feature width (512)
N_CORES = 8
REPLICA_GROUPS = [[2 * b, 2 * b + 1] for b in range(B)]


def _make_pools(tc, ctx):
    p = {}
    p["consts"] = ctx.enter_context(tc.tile_pool(name="consts", bufs=1))
    p["pp"] = ctx.enter_context(tc.tile_pool(name="pp", bufs=6))
    p["rp"] = ctx.enter_context(tc.tile_pool(name="rp", bufs=4))
    p["ft"] = ctx.enter_context(tc.tile_pool(name="ft", bufs=3))
    p["psMM"] = ctx.enter_context(tc.tile_pool(name="psMM", bufs=2, space="PSUM"))
    p["psY"] = ctx.enter_context(tc.tile_pool(name="psY", bufs=4, space="PSUM"))
    p["dram"] = ctx.enter_context(tc.tile_pool(name="dram", bufs=1, space="DRAM"))
    return p


def _segments(qoff, width):
    """Split [qoff, width) at multiples of 512 (PSUM bank boundary)."""
    segs = []
    a = qoff
    while a < width:
        b = min((a // 512 + 1) * 512, width)
        segs.append((a, b))
        a = b
    return segs


def _emit_body(nc, tc, p, io, t_seq, collective=True, upto='E'):
    """Emit one full forward pass. t_seq: sequence length (2048, or smaller for sim)."""
    CT = C // 128          # contraction tiles (8)
    TT = t_seq // 128      # token 128-tiles
    TJ = t_seq // 1024     # token 1024-chunks
    FT = FL // 128         # local f 128-tiles for q/k (4)
    PT = FL // 128         # wp c_loc tiles (4)

    consts = p["consts"]

    # ---- constants ----
    ident_b = consts.tile([128, 128], BF16, tag="identb")
    make_identity(nc, ident_b)
    negtri = consts.tile([128, 128], BF16, tag="negtri")
    make_upper_triangular(nc, negtri, val=-50.0, diag=False)

    bq_t = consts.tile([128, FT], F32, tag="bq")
    nc.sync.dma_start(bq_t, io["bqs"].rearrange("(j p) -> p j", p=128))
    bk_t = consts.tile([128, FT], F32, tag="bk")
    nc.sync.dma_start(bk_t, io["bk"].rearrange("(j p) -> p j", p=128))
    bv_f = consts.tile([1, FL], F32, tag="bvf")
    nc.sync.dma_start(bv_f, io["bv"].rearrange("(a f) -> a f", a=1))
    bp_f = consts.tile([1, C], F32, tag="bpf")
    nc.sync.dma_start(bp_f, io["bph"].rearrange("(a f) -> a f", a=1))
    bv_bc = consts.tile([128, FL], F32, tag="bvbc")
    nc.gpsimd.partition_broadcast(bv_bc, bv_f)
    bp_bc = consts.tile([128, C], F32, tag="bpbc")
    nc.gpsimd.partition_broadcast(bp_bc, bp_f)

    # ---- persistent tiles (split for fine-grained deps) ----
    xT = [consts.tile([128, t_seq], BF16, tag=f"xT{i}", name=f"xT{i}") for i in range(CT)]
    wqTs = [consts.tile([128, FL], BF16, tag=f"wqT{i}", name=f"wqT{i}") for i in range(CT)]
    wkTs = [consts.tile([128, FL], BF16, tag=f"wkT{i}", name=f"wkT{i}") for i in range(CT)]
    wvTs = [consts.tile([128, FL], BF16, tag=f"wvT{i}", name=f"wvT{i}") for i in range(CT)]
    wpTs = [consts.tile([128, C], BF16, tag=f"wpT{i}", name=f"wpT{i}") for i in range(PT)]
    QT = [consts.tile([128, t_seq], BF16, tag=f"QT{i}", name=f"QT{i}") for i in range(FT)]
    KT = [consts.tile([128, t_seq], BF16, tag=f"KT{i}", name=f"KT{i}") for i in range(FT)]
    Vt = [consts.tile([128, HL * 65], BF16, tag=f"Vt{i}", name=f"Vt{i}") for i in range(TT)]
    yT = [consts.tile([128, FT, 1024], BF16, tag=f"yT{i}", name=f"yT{i}") for i in range(TJ)]

    for tt in range(TT):  # ones columns of V
        nc.vector.memset(Vt[tt].rearrange("p (h e) -> p h e", h=HL)[:, :, 64:65], 1.0)

    # ---- phase A: inputs arrive bf16; hardware DMA-transpose straight to SBUF ----
    # Interleave x/wq/wk per-ct so the first Q matmul chain can start after the
    # first few transfers rather than after the whole load.
    for ct in range(CT):
        nc.sync.dma_start(xT[ct], io["x"][:, ct * 128:(ct + 1) * 128], transpose=True)
        nc.sync.dma_start(wqTs[ct], io["wq"][:, ct * 128:(ct + 1) * 128], transpose=True)
        nc.sync.dma_start(wkTs[ct], io["wk"][:, ct * 128:(ct + 1) * 128], transpose=True)
    for ct in range(CT):
        nc.sync.dma_start(wvTs[ct], io["wv"][:, ct * 128:(ct + 1) * 128], transpose=True)
    for ci in range(PT):
        nc.sync.dma_start(wpTs[ci], io["wp"][:, ci * 128:(ci + 1) * 128], transpose=True)

    cc_in = p["dram"].tile([t_seq, C], BF16, tag="cc_in")

    if upto == 'A':
        nc.gpsimd.dma_start(io["out"], cc_in[0:t_seq // 2, :])
        return

    # ---- phase B pieces: QKV projection ----
    def emit_qk(ft):
        # Q^T[f, t] = (sum_c wqT[c, f] xT[c, t] + bq) / 8; K^T likewise (no scale)
        for dst, w_T, bias, scl in ((QT, wqTs, bq_t, 0.125), (KT, wkTs, bk_t, None)):
            for tj in range(TJ):
                ps = p["psMM"].tile([128, 1024], F32, tag="psMM", name="psMM")
                for half in range(2):
                    for ct in range(CT):
                        nc.tensor.matmul(
                            ps[:, half * 512:(half + 1) * 512],
                            lhsT=w_T[ct][:, ft * 128:(ft + 1) * 128],
                            rhs=xT[ct][:, tj * 1024 + half * 512: tj * 1024 + (half + 1) * 512],
                            start=(ct == 0),
                            stop=(ct == CT - 1),
                        )
                if scl is None:
                    nc.vector.tensor_scalar(
                        dst[ft][:, tj * 1024:(tj + 1) * 1024], ps,
                        scalar1=bias[:, ft:ft + 1], scalar2=None, op0=ALU.add,
                    )
                else:
                    nc.vector.tensor_scalar(
                        dst[ft][:, tj * 1024:(tj + 1) * 1024], ps,
                        scalar1=bias[:, ft:ft + 1], scalar2=scl,
                        op0=ALU.add, op1=ALU.mult,
                    )

    def emit_v(tts):
        # V[t, f] = sum_c xT[c, t] wvT[c, f] + bv (bias via broadcast add on DVE)
        for tt in tts:
            ps = p["psMM"].tile([128, 1024], F32, tag="psMM", name="psMM")
            for ct in range(CT):
                nc.tensor.matmul(
                    ps[:, 0:512],
                    lhsT=xT[ct][:, tt * 128:(tt + 1) * 128],
                    rhs=wvTs[ct],
                    start=(ct == 0),
                    stop=(ct == CT - 1),
                )
            nc.vector.tensor_add(
                Vt[tt].rearrange("p (h e) -> p h e", h=HL)[:, :, 0:64],
                ps[:, 0:512].rearrange("p (h e) -> p h e", h=HL),
                bv_bc.rearrange("p (h e) -> p h e", h=HL),
            )

    # ---- phase C: attention, 1024-wide q-chunks, two heads interleaved,
    # AV delayed one k-step behind S/exp to hide activation latency ----
    def emit_c(jq, hp):
        ctxs = []
        for h in (hp, hp + 1):
            ctxs.append({
                "h": h, "ftq": h // 2, "po": (h % 2) * 64,
                "ypA": p["psY"].tile([65, 512], F32, tag="psY", name=f"ypA{h}"),
                "ypB": p["psY"].tile([65, 512], F32, tag="psY", name=f"ypB{h}"),
            })
        ni = 8 * (jq + 1)
        last_a = 8 * jq + 3        # last k-tile writing cols [0,512)

        def emit_s(cx, i):
            q0 = max(jq * 1024, i * 128)
            qoff = q0 - jq * 1024
            diag = i * 128 >= jq * 1024
            ftq, po = cx["ftq"], cx["po"]
            sp = p["psMM"].tile([128, 1024], F32, tag="psMM", name="psMM")
            for si, (a, b) in enumerate(_segments(qoff, 1024)):
                nc.tensor.matmul(
                    sp[:, a:b],
                    lhsT=KT[ftq][po:po + 64, i * 128:(i + 1) * 128],
                    rhs=QT[ftq][po:po + 64, jq * 1024 + a: jq * 1024 + b],
                    start=True,
                    stop=not (diag and si == 0),
                )
            if diag:
                # causal mask: add -50 to q<k of the diagonal block
                nc.tensor.matmul(
                    sp[:, qoff:qoff + 128], lhsT=negtri, rhs=ident_b,
                    start=False, stop=True,
                )
            pt = p["pp"].tile([128, 1024], BF16, tag="pp", name="pp")
            nc.scalar.activation(pt[:, qoff:1024], sp[:, qoff:1024], AF.Exp)
            return pt, qoff

        def emit_av(cx, i, pt, qoff):
            for (a, b) in _segments(qoff, 1024):
                yp, off, lst = (
                    (cx["ypA"], 0, last_a) if a < 512 else (cx["ypB"], 512, ni - 1)
                )
                nc.tensor.matmul(
                    yp[:, a - off:b - off],
                    lhsT=Vt[i][:, cx["h"] * 65:cx["h"] * 65 + 65],
                    rhs=pt[:, a:b],
                    start=(i == 0),
                    stop=(i == lst),
                )

        pend = None
        for i in range(ni):
            res = [(cx, emit_s(cx, i)) for cx in ctxs]
            if pend is not None:
                for cx, (pt, qoff) in pend:
                    emit_av(cx, pend_i, pt, qoff)
            pend, pend_i = res, i
        for cx, (pt, qoff) in pend:
            emit_av(cx, pend_i, pt, qoff)

        # normalize: yT = yp[0:64] / yp[64]; copy out of PSUM first so the
        # accumulator slot frees after one DVE op, not the whole chain
        for cx in ctxs:
            ftq, po = cx["ftq"], cx["po"]
            for yp, off in ((cx["ypA"], 0), (cx["ypB"], 512)):
                yc = p["rp"].tile([65, 512], F32, tag="yc", name="yc")
                nc.vector.tensor_copy(yc, yp)
                r = p["rp"].tile([1, 512], F32, tag="r", name="r")
                nc.vector.reciprocal(r, yc[64:65, :])
                R = p["rp"].tile([64, 512], F32, tag="R", name="R")
                nc.gpsimd.partition_broadcast(R, r)
                nc.vector.tensor_mul(
                    yT[jq][po:po + 64, ftq, off:off + 512], yc[0:64, :], R
                )

    # ---- phase D: output projection, token-major (t on PSUM partitions) ----
    def emit_d(tj, tts):
        for tt in tts:
            ps = p["psMM"].tile([128, 1024], F32, tag="psMM", name="psMM")
            for half in range(2):
                for ci in range(PT):
                    nc.tensor.matmul(
                        ps[:, half * 512:(half + 1) * 512],
                        lhsT=yT[tj][:, ci, tt * 128:(tt + 1) * 128],
                        rhs=wpTs[ci][:, half * 512:(half + 1) * 512],
                        start=(ci == 0),
                        stop=(ci == PT - 1),
                    )
            ob = p["ft"].tile([128, 1024], BF16, tag="ob", name="ob")
            nc.vector.tensor_add(ob, ps, bp_bc)
            row = tj * 1024 + tt * 128
            nc.sync.dma_start(cc_in[row:row + 128, :], ob)

    # ---- emission schedule: interleave B / C / D so PE never starves ----
    emit_qk(0)
    emit_v(range(0, min(8, TT)))
    if upto == 'B':
        # finish everything B-ish for a fair partial sim
        for ft in range(1, FT):
            emit_qk(ft)
        emit_v(range(8, TT))
        nc.gpsimd.dma_start(io["out"], cc_in[0:t_seq // 2, :])
        return
    emit_c(0, 0)
    emit_qk(1)
    emit_c(0, 2)
    emit_v(range(8, TT))
    emit_qk(2)
    emit_c(0, 4)
    emit_qk(3)
    emit_c(0, 6)
    if TJ > 1:
        emit_c(1, 0)
        emit_d(0, range(0, 4))
        emit_c(1, 2)
        emit_d(0, range(4, 8))
        emit_c(1, 4)
        emit_c(1, 6)
    else:
        emit_d(0, range(0, 8))
    if upto == 'C':
        nc.gpsimd.dma_start(io["out"], cc_in[0:t_seq // 2, :])
        return

    # ---- phase E: chunked pairwise ReduceScatter + store ----
    hc = t_seq // 2   # tokens per collective chunk (half of t_seq)
    oc = t_seq // 4   # output rows per chunk per core
    if collective:
        cc_out = p["dram"].tile([2 * oc, C], BF16, tag="cc_out")
        nc.gpsimd.collective_compute(
            "ReduceScatter",
            ALU.add,
            replica_groups=REPLICA_GROUPS,
            ins=[cc_in[0:hc, :].opt()],
            outs=[cc_out[0:oc, :].opt()],
        )
        nc.gpsimd.dma_start(io["out"][0:oc, :], cc_out[0:oc, :])
        if TJ > 1:
            emit_d(1, range(0, 8))
            nc.gpsimd.collective_compute(
                "ReduceScatter",
                ALU.add,
                replica_groups=REPLICA_GROUPS,
                ins=[cc_in[hc:t_seq, :].opt()],
                outs=[cc_out[oc:2 * oc, :].opt()],
            )
            nc.gpsimd.dma_start(io["out"][oc:2 * oc, :], cc_out[oc:2 * oc, :])
    else:
        if TJ > 1:
            emit_d(1, range(0, 8))
        nc.gpsimd.dma_start(io["out"], cc_in[0:t_seq // 2, :])


def build_program(t_seq=T, repeat=1, collective=True, upto='E'):
    nc = bacc.Bacc("TRN2", target_bir_lowering=False, debug=False, num_devices=N_CORES)
    io = {
        "x": nc.dram_tensor("x", [t_seq, C], BF16, kind="ExternalInput").ap(),
        "wq": nc.dram_tensor("wq", [FL, C], BF16, kind="ExternalInput").ap(),
        "wk": nc.dram_tensor("wk", [FL, C], BF16, kind="ExternalInput").ap(),
        "wv": nc.dram_tensor("wv", [FL, C], BF16, kind="ExternalInput").ap(),
        "wp": nc.dram_tensor("wp", [C, FL], BF16, kind="ExternalInput").ap(),
        "bqs": nc.dram_tensor("bqs", [FL], F32, kind="ExternalInput").ap(),
        "bk": nc.dram_tensor("bk", [FL], F32, kind="ExternalInput").ap(),
        "bv": nc.dram_tensor("bv", [FL], F32, kind="ExternalInput").ap(),
        "bph": nc.dram_tensor("bph", [C], F32, kind="ExternalInput").ap(),
        "out": nc.dram_tensor("out", [t_seq // 2, C], BF16, kind="ExternalOutput").ap(),
    }
    with tile.TileContext(nc) as tc:
        with ExitStack() as ctx:
            pools = _make_pools(tc, ctx)
            if repeat == 1:
                _emit_body(nc, tc, pools, io, t_seq, collective=collective, upto=upto)
            else:
                with tc.For_i(0, repeat, 1) as _:
                    _emit_body(nc, tc, pools, io, t_seq, collective=collective, upto=upto)
    nc.compile()
    return nc


def make_in_maps(x, w_attn, b_attn, w_proj, b_proj):
    x = np.ascontiguousarray(np.asarray(x, dtype=np.float32))
    w_attn = np.asarray(w_attn, dtype=np.float32)
    b_attn = np.asarray(b_attn, dtype=np.float32)
    w_proj = np.asarray(w_proj, dtype=np.float32)
    b_proj = np.asarray(b_proj, dtype=np.float32)
    in_maps = []
    for c in range(N_CORES):
        b, g = c // 2, c % 2
        fs = slice(g * FL, (g + 1) * FL)
        bf = ml_dtypes.bfloat16
        in_maps.append({
            "x": x[b].astype(bf),
            "wq": np.ascontiguousarray(w_attn[0 * C:][:C][fs]).astype(bf),
            "wk": np.ascontiguousarray(w_attn[1 * C:][:C][fs]).astype(bf),
            "wv": np.ascontiguousarray(w_attn[2 * C:][:C][fs]).astype(bf),
            "wp": np.ascontiguousarray(w_proj[:, fs]).astype(bf),
            "bqs": np.ascontiguousarray(b_attn[0 * C:][:C][fs]),
            "bk": np.ascontiguousarray(b_attn[1 * C:][:C][fs]),
            "bv": np.ascontiguousarray(b_attn[2 * C:][:C][fs]),
            "bph": b_proj * np.float32(0.5),
        })
    return in_maps


_PROG = None


def kernel(x, w_attn, b_attn, w_proj, b_proj):
    global _PROG
    if _PROG is None:
        _PROG = build_program()
    in_maps = make_in_maps(x, w_attn, b_attn, w_proj, b_proj)
    res = run_bass_kernel_spmd(_PROG, in_maps, core_ids=list(range(N_CORES))).results
    out = np.empty((B, T, C), dtype=np.float32)
    Q = T // 4  # rows per collective chunk per core (512)
    for c in range(N_CORES):
        b, g = c // 2, c % 2
        o = res[c]["out"].astype(np.float32)
        # chunk m covers tokens [m*T/2, (m+1)*T/2); core g gets its half
        for m in range(2):
            t0 = m * (T // 2) + g * Q
            out[b, t0:t0 + Q, :] = o[m * Q:(m + 1) * Q]
    return out


# revision 9
# speedup vs baseline: 35.1821x; 35.1821x over previous
"""Trainium2 Bass kernel for causal self-attention (B=4, T=2048, C=1024, H=16).

Sharding: 8 cores = 4 batches (data-parallel) x 2 head-groups (tensor-parallel,
8 heads each). Each core computes QKV for its heads, flash-style causal
attention, and a partial output projection over its half of the channels;
a pairwise ReduceScatter over tokens combines the two partials per batch.

Self-contained: hardcodes shapes; host side only slices/concats numpy arrays.
"""

import ml_dtypes
import numpy as np
from contextlib import ExitStack

import concourse.bass as bass
import concourse.tile as tile
from concourse import bacc, mybir
from concourse.bass_utils import run_bass_kernel_spmd
from concourse.masks import make_identity, make_upper_triangular

F32 = mybir.dt.float32
BF16 = mybir.dt.bfloat16
AF = mybir.ActivationFunctionType
ALU = mybir.AluOpType

B, T, C = 4, 2048, 1024
H, HD = 16, 64
G = 2                    # tensor-parallel head groups
HL = H // G              # heads per core (8)
FL = HL * HD             # local q/k/v feature width (512)
N_CORES = 8
REPLICA_GROUPS = [[2 * b, 2 * b + 1] for b in range(B)]


def _make_pools(tc, ctx):
    p = {}
    p["consts"] = ctx.enter_context(tc.tile_pool(name="consts", bufs=1))
    p["pp"] = ctx.enter_context(tc.tile_pool(name="pp", bufs=4))
    p["rp"] = ctx.enter_context(tc.tile_pool(name="rp", bufs=3))
    p["ft"] = ctx.enter_context(tc.tile_pool(name="ft", bufs=3))
    p["psMM"] = ctx.enter_context(tc.tile_pool(name="psMM", bufs=2, space="PSUM"))
    p["psY"] = ctx.enter_context(tc.tile_pool(name="psY", bufs=4, space="PSUM"))
    p["dram"] = ctx.enter_context(tc.tile_pool(name="dram", bufs=1, space="DRAM"))
    return p


def _segments(qoff, width):
    """Split [qoff, width) at multiples of 512 (PSUM bank boundary)."""
    segs = []
    a = qoff
    while a < width:
        b = min((a // 512 + 1) * 512, width)
        segs.append((a, b))
        a = b
    return segs


def _emit_body(nc, tc, p, io, t_seq, collective=True, upto='E'):
    """Emit one full forward pass. t_seq: sequence length (2048, or smaller for sim)."""
    CT = C // 128          # contraction tiles (8)
    TT = t_seq // 128      # token 128-tiles
    TJ = t_seq // 1024     # token 1024-chunks
    FT = FL // 128         # local f 128-tiles for q/k (4)
    PT = FL // 128         # wp c_loc tiles (4)

    consts = p["consts"]

    # ---- constants ----
    ident_b = consts.tile([128, 128], BF16, tag="identb")
    make_identity(nc, ident_b)
    negtri = consts.tile([128, 128], BF16, tag="negtri")
    make_upper_triangular(nc, negtri, val=-50.0, diag=False)

    bq_t = consts.tile([128, FT], F32, tag="bq")
    nc.sync.dma_start(bq_t, io["bqs"].rearrange("(j p) -> p j", p=128))
    bk_t = consts.tile([128, FT], F32, tag="bk")
    nc.sync.dma_start(bk_t, io["bk"].rearrange("(j p) -> p j", p=128))
    bv_f = consts.tile([1, FL], F32, tag="bvf")
    nc.sync.dma_start(bv_f, io["bv"].rearrange("(a f) -> a f", a=1))
    bp_f = consts.tile([1, C], F32, tag="bpf")
    nc.sync.dma_start(bp_f, io["bph"].rearrange("(a f) -> a f", a=1))
    bv_bc = consts.tile([128, FL], F32, tag="bvbc")
    nc.gpsimd.partition_broadcast(bv_bc, bv_f)
    bp_bc = consts.tile([128, C], F32, tag="bpbc")
    nc.gpsimd.partition_broadcast(bp_bc, bp_f)

    # ---- persistent tiles (split for fine-grained deps) ----
    xT = [consts.tile([128, t_seq], BF16, tag=f"xT{i}", name=f"xT{i}") for i in range(CT)]
    wqTs = [consts.tile([128, FL], BF16, tag=f"wqT{i}", name=f"wqT{i}") for i in range(CT)]
    wkTs = [consts.tile([128, FL], BF16, tag=f"wkT{i}", name=f"wkT{i}") for i in range(CT)]
    wvTs = [consts.tile([128, FL], BF16, tag=f"wvT{i}", name=f"wvT{i}") for i in range(CT)]
    wpTs = [consts.tile([128, C], BF16, tag=f"wpT{i}", name=f"wpT{i}") for i in range(PT)]
    # per-head Q/K with head-dim duplicated to 128 partitions: K=128 matmuls
    # run at full (double-pumped) rate on HW, K=64 only at half rate.
    QT2 = [consts.tile([128, t_seq], BF16, tag=f"QT2{i}", name=f"QT2{i}") for i in range(HL)]
    KT2 = [consts.tile([128, t_seq], BF16, tag=f"KT2{i}", name=f"KT2{i}") for i in range(HL)]
    Vt = [consts.tile([128, HL * 65], BF16, tag=f"Vt{i}", name=f"Vt{i}") for i in range(TT)]
    yT = [consts.tile([128, FT, 1024], BF16, tag=f"yT{i}", name=f"yT{i}") for i in range(TJ)]

    for tt in range(TT):  # ones columns of V
        nc.vector.memset(Vt[tt].rearrange("p (h e) -> p h e", h=HL)[:, :, 64:65], 1.0)

    # ---- phase A: inputs arrive bf16 and pre-transposed on the host, so all
    # loads are contiguous (DmaTranspose is far slower than modeled on HW) ----
    for ct in range(CT):
        nc.sync.dma_start(xT[ct], io["x"][ct * 128:(ct + 1) * 128, :])
        nc.sync.dma_start(wqTs[ct], io["wq"][ct * 128:(ct + 1) * 128, :])
        nc.sync.dma_start(wkTs[ct], io["wk"][ct * 128:(ct + 1) * 128, :])
    for ct in range(CT):
        nc.sync.dma_start(wvTs[ct], io["wv"][ct * 128:(ct + 1) * 128, :])
    for ci in range(PT):
        nc.sync.dma_start(wpTs[ci], io["wp"][ci * 128:(ci + 1) * 128, :])

    cc_in = p["dram"].tile([t_seq, C], BF16, tag="cc_in")

    if upto == 'A':
        nc.gpsimd.dma_start(io["out"], cc_in[0:t_seq // 2, :])
        return

    # ---- phase B pieces: QKV projection ----
    def emit_qk(ft):
        # Q^T[f, t] = (sum_c wqT[c, f] xT[c, t] + bq) / 16; K^T likewise
        # (no scale). Q scale is 1/16 not 1/8: the head-dim duplication in
        # QT2/KT2 double-counts the contraction.
        h0, h1 = 2 * ft, 2 * ft + 1
        for dst, w_T, bias, scl in ((QT2, wqTs, bq_t, 0.0625), (KT2, wkTs, bk_t, None)):
            for tj in range(TJ):
                tsl = slice(tj * 1024, (tj + 1) * 1024)
                ps = p["psMM"].tile([128, 1024], F32, tag="psMM", name="psMM")
                for half in range(2):
                    for ct in range(CT):
                        nc.tensor.matmul(
                            ps[:, half * 512:(half + 1) * 512],
                            lhsT=w_T[ct][:, ft * 128:(ft + 1) * 128],
                            rhs=xT[ct][:, tj * 1024 + half * 512: tj * 1024 + (half + 1) * 512],
                            start=(ct == 0),
                            stop=(ct == CT - 1),
                        )
                for h, pr, wr in ((h0, slice(0, 64), slice(64, 128)),
                                  (h1, slice(64, 128), slice(0, 64))):
                    if scl is None:
                        nc.vector.tensor_scalar(
                            dst[h][pr, tsl], ps[pr, :],
                            scalar1=bias[pr, ft:ft + 1], scalar2=None, op0=ALU.add,
                        )
                    else:
                        nc.vector.tensor_scalar(
                            dst[h][pr, tsl], ps[pr, :],
                            scalar1=bias[pr, ft:ft + 1], scalar2=scl,
                            op0=ALU.add, op1=ALU.mult,
                        )
                    # duplicate the 64 head features into the other 64
                    # partitions (partition shift -> DMA)
                    nc.sync.dma_start(dst[h][wr, tsl], dst[h][pr, tsl])

    def emit_v(tts):
        # V[t, f] = sum_c xT[c, t] wvT[c, f] + bv (bias via broadcast add on DVE)
        for tt in tts:
            ps = p["psMM"].tile([128, 1024], F32, tag="psMM", name="psMM")
            for ct in range(CT):
                nc.tensor.matmul(
                    ps[:, 0:512],
                    lhsT=xT[ct][:, tt * 128:(tt + 1) * 128],
                    rhs=wvTs[ct],
                    start=(ct == 0),
                    stop=(ct == CT - 1),
                )
            nc.vector.tensor_add(
                Vt[tt].rearrange("p (h e) -> p h e", h=HL)[:, :, 0:64],
                ps[:, 0:512].rearrange("p (h e) -> p h e", h=HL),
                bv_bc.rearrange("p (h e) -> p h e", h=HL),
            )

    # ---- phase C: attention, 1024-wide q-chunks, two heads interleaved,
    # AV delayed one k-step behind S/exp to hide activation latency ----
    def emit_c(jq, hp):
        ctxs = []
        for h in (hp, hp + 1):
            ctxs.append({
                "h": h, "ftq": h // 2, "po": (h % 2) * 64,
                "ypA": p["psY"].tile([65, 512], F32, tag="psY", name=f"ypA{h}"),
                "ypB": p["psY"].tile([65, 512], F32, tag="psY", name=f"ypB{h}"),
            })
        ni = 8 * (jq + 1)
        last_a = 8 * jq + 3        # last k-tile writing cols [0,512)

        def emit_s(cx, i):
            q0 = max(jq * 1024, i * 128)
            qoff = q0 - jq * 1024
            diag = i * 128 >= jq * 1024
            h = cx["h"]
            sp = p["psMM"].tile([128, 1024], F32, tag="psMM", name="psMM")
            for si, (a, b) in enumerate(_segments(qoff, 1024)):
                nc.tensor.matmul(
                    sp[:, a:b],
                    lhsT=KT2[h][:, i * 128:(i + 1) * 128],
                    rhs=QT2[h][:, jq * 1024 + a: jq * 1024 + b],
                    start=True,
                    stop=not (diag and si == 0),
                )
            if diag:
                # causal mask: add -50 to q<k of the diagonal block
                nc.tensor.matmul(
                    sp[:, qoff:qoff + 128], lhsT=negtri, rhs=ident_b,
                    start=False, stop=True,
                )
            pt = p["pp"].tile([128, 1024], BF16, tag="pp", name="pp")
            nc.scalar.activation(pt[:, qoff:1024], sp[:, qoff:1024], AF.Exp)
            return pt, qoff

        def emit_av(cx, i, pt, qoff):
            for (a, b) in _segments(qoff, 1024):
                yp, off, lst = (
                    (cx["ypA"], 0, last_a) if a < 512 else (cx["ypB"], 512, ni - 1)
                )
                nc.tensor.matmul(
                    yp[:, a - off:b - off],
                    lhsT=Vt[i][:, cx["h"] * 65:cx["h"] * 65 + 65],
                    rhs=pt[:, a:b],
                    start=(i == 0),
                    stop=(i == lst),
                )

        pend = None
        for i in range(ni):
            res = [(cx, emit_s(cx, i)) for cx in ctxs]
            if pend is not None:
                for cx, (pt, qoff) in pend:
                    emit_av(cx, pend_i, pt, qoff)
            pend, pend_i = res, i
        for cx, (pt, qoff) in pend:
            emit_av(cx, pend_i, pt, qoff)

        # normalize: yT = yp[0:64] / yp[64]; copy out of PSUM first so the
        # accumulator slot frees after one DVE op, not the whole chain
        for cx in ctxs:
            ftq, po = cx["ftq"], cx["po"]
            for yp, off in ((cx["ypA"], 0), (cx["ypB"], 512)):
                yc = p["rp"].tile([65, 512], F32, tag="yc", name="yc")
                nc.vector.tensor_copy(yc, yp)
                r = p["rp"].tile([1, 512], F32, tag="r", name="r")
                nc.vector.reciprocal(r, yc[64:65, :])
                R = p["rp"].tile([64, 512], F32, tag="R", name="R")
                nc.gpsimd.partition_broadcast(R, r)
                nc.vector.tensor_mul(
                    yT[jq][po:po + 64, ftq, off:off + 512], yc[0:64, :], R
                )

    # ---- phase D: output projection, token-major (t on PSUM partitions) ----
    def emit_d(tj, tts):
        for tt in tts:
            ps = p["psMM"].tile([128, 1024], F32, tag="psMM", name="psMM")
            for half in range(2):
                for ci in range(PT):
                    nc.tensor.matmul(
                        ps[:, half * 512:(half + 1) * 512],
                        lhsT=yT[tj][:, ci, tt * 128:(tt + 1) * 128],
                        rhs=wpTs[ci][:, half * 512:(half + 1) * 512],
                        start=(ci == 0),
                        stop=(ci == PT - 1),
                    )
            ob = p["ft"].tile([128, 1024], BF16, tag="ob", name="ob")
            nc.vector.tensor_add(ob, ps, bp_bc)
            row = tj * 1024 + tt * 128
            nc.sync.dma_start(cc_in[row:row + 128, :], ob)

    # ---- emission schedule: interleave B / C / D so PE never starves ----
    emit_qk(0)
    emit_v(range(0, min(8, TT)))
    if upto == 'B':
        # finish everything B-ish for a fair partial sim
        for ft in range(1, FT):
            emit_qk(ft)
        emit_v(range(8, TT))
        nc.gpsimd.dma_start(io["out"], cc_in[0:t_seq // 2, :])
        return
    emit_c(0, 0)
    emit_qk(1)
    emit_c(0, 2)
    emit_v(range(8, TT))
    emit_qk(2)
    emit_c(0, 4)
    emit_qk(3)
    emit_c(0, 6)
    if TJ > 1:
        emit_c(1, 0)
        emit_d(0, range(0, 4))
        emit_c(1, 2)
        emit_d(0, range(4, 8))
        emit_c(1, 4)
        emit_c(1, 6)
    else:
        emit_d(0, range(0, 8))
    if upto == 'C':
        nc.gpsimd.dma_start(io["out"], cc_in[0:t_seq // 2, :])
        return

    # ---- phase E: chunked pairwise ReduceScatter + store ----
    hc = t_seq // 2   # tokens per collective chunk (half of t_seq)
    oc = t_seq // 4   # output rows per chunk per core
    if collective:
        cc_out = p["dram"].tile([2 * oc, C], BF16, tag="cc_out")
        nc.gpsimd.collective_compute(
            "ReduceScatter",
            ALU.add,
            replica_groups=REPLICA_GROUPS,
            ins=[cc_in[0:hc, :].opt()],
            outs=[cc_out[0:oc, :].opt()],
        )
        nc.gpsimd.dma_start(io["out"][0:oc, :], cc_out[0:oc, :])
        if TJ > 1:
            emit_d(1, range(0, 8))
            nc.gpsimd.collective_compute(
                "ReduceScatter",
                ALU.add,
                replica_groups=REPLICA_GROUPS,
                ins=[cc_in[hc:t_seq, :].opt()],
                outs=[cc_out[oc:2 * oc, :].opt()],
            )
            nc.gpsimd.dma_start(io["out"][oc:2 * oc, :], cc_out[oc:2 * oc, :])
    else:
        if TJ > 1:
            emit_d(1, range(0, 8))
        nc.gpsimd.dma_start(io["out"], cc_in[0:t_seq // 2, :])


def build_program(t_seq=T, repeat=1, collective=True, upto='E'):
    nc = bacc.Bacc("TRN2", target_bir_lowering=False, debug=False, num_devices=N_CORES)
    io = {
        # all pre-transposed host-side: x [C, T], w* [C_in, F_out]
        "x": nc.dram_tensor("x", [C, t_seq], BF16, kind="ExternalInput").ap(),
        "wq": nc.dram_tensor("wq", [C, FL], BF16, kind="ExternalInput").ap(),
        "wk": nc.dram_tensor("wk", [C, FL], BF16, kind="ExternalInput").ap(),
        "wv": nc.dram_tensor("wv", [C, FL], BF16, kind="ExternalInput").ap(),
        "wp": nc.dram_tensor("wp", [FL, C], BF16, kind="ExternalInput").ap(),
        "bqs": nc.dram_tensor("bqs", [FL], F32, kind="ExternalInput").ap(),
        "bk": nc.dram_tensor("bk", [FL], F32, kind="ExternalInput").ap(),
        "bv": nc.dram_tensor("bv", [FL], F32, kind="ExternalInput").ap(),
        "bph": nc.dram_tensor("bph", [C], F32, kind="ExternalInput").ap(),
        "out": nc.dram_tensor("out", [t_seq // 2, C], BF16, kind="ExternalOutput").ap(),
    }
    with tile.TileContext(nc) as tc:
        with ExitStack() as ctx:
            pools = _make_pools(tc, ctx)
            if repeat == 1:
                _emit_body(nc, tc, pools, io, t_seq, collective=collective, upto=upto)
            else:
                with tc.For_i(0, repeat, 1) as _:
                    _emit_body(nc, tc, pools, io, t_seq, collective=collective, upto=upto)
    nc.compile()
    return nc


def make_in_maps(x, w_attn, b_attn, w_proj, b_proj):
    x = np.ascontiguousarray(np.asarray(x, dtype=np.float32))
    w_attn = np.asarray(w_attn, dtype=np.float32)
    b_attn = np.asarray(b_attn, dtype=np.float32)
    w_proj = np.asarray(w_proj, dtype=np.float32)
    b_proj = np.asarray(b_proj, dtype=np.float32)
    in_maps = []
    for c in range(N_CORES):
        b, g = c // 2, c % 2
        fs = slice(g * FL, (g + 1) * FL)
        bf = ml_dtypes.bfloat16
        in_maps.append({
            "x": np.ascontiguousarray(x[b].T).astype(bf),
            "wq": np.ascontiguousarray(w_attn[0 * C:][:C][fs].T).astype(bf),
            "wk": np.ascontiguousarray(w_attn[1 * C:][:C][fs].T).astype(bf),
            "wv": np.ascontiguousarray(w_attn[2 * C:][:C][fs].T).astype(bf),
            "wp": np.ascontiguousarray(w_proj[:, fs].T).astype(bf),
            "bqs": np.ascontiguousarray(b_attn[0 * C:][:C][fs]),
            "bk": np.ascontiguousarray(b_attn[1 * C:][:C][fs]),
            "bv": np.ascontiguousarray(b_attn[2 * C:][:C][fs]),
            "bph": b_proj * np.float32(0.5),
        })
    return in_maps


_PROG = None


def kernel(x, w_attn, b_attn, w_proj, b_proj):
    global _PROG
    if _PROG is None:
        _PROG = build_program()
    in_maps = make_in_maps(x, w_attn, b_attn, w_proj, b_proj)
    res = run_bass_kernel_spmd(_PROG, in_maps, core_ids=list(range(N_CORES))).results
    out = np.empty((B, T, C), dtype=np.float32)
    Q = T // 4  # rows per collective chunk per core (512)
    for c in range(N_CORES):
        b, g = c // 2, c % 2
        o = res[c]["out"].astype(np.float32)
        # chunk m covers tokens [m*T/2, (m+1)*T/2); core g gets its half
        for m in range(2):
            t0 = m * (T // 2) + g * Q
            out[b, t0:t0 + Q, :] = o[m * Q:(m + 1) * Q]
    return out
